# revision 2
# baseline (speedup 1.0000x reference)
"""Causal attention (K Q^T variant) on 8 Trainium2 NeuronCores.

Math (identical reduction to the original baseline):
    S_raw^T[s,t] = sum_c M^T[c,s] x^T[c,t] + b[s],  M = x (wq^T wk)
    P^T = exp(S_raw^T/sqrt(C)) causally masked; out = (P^T)^T V / rowsum,
    V = x wv^T + bv.
  a[t]-type bias terms and bk.bq cancel in the softmax; b = x (wq^T bk)
  rides in the exp bias; both G = wq^T wk and b are host-precomputed.

Precision plan (gate: rel err < 2e-2; this kernel measures ~1.83e-2):
  - M projection AND scores GEMM run in fp8(e4m3) DoubleRow: 256-feature
    contraction per matmul, twice the fp16 rate. G ships pre-scaled by 64
    and M is stored pre-scaled by 8 so fp8 values avoid the subnormal
    range; the scales fold into the PSUM->SBUF copy and the exp scale.
  - V projection and the AV GEMM stay fp16: their quantization error
    passes straight to the output (fp8 there measures ~5e-2).

Schedule (PE-serial phases; everything else overlaps):
  warmup matmuls (HAM clock ramp) -> M projection (fp8, four T-quarter
  passes over 8 one-bank PSUM tiles) -> scores+exp (fp8, needs only
  x8/MT8/bias, so it runs right after the projection while x/wv still
  stream) -> V projection (fp16) -> AV (fp16, j descending so the kernel
  tail ends on the smallest block; ones-column in V gives the softmax
  denominator in the same accumulation).
DMA: ring 1 (sync) carries x8 then x(fp16); ring 2 (scalar) carries
G8/biases/wv. Descriptors are sized so arrival order matches consumption
order with wide margins.
"""

import numpy as np

import concourse.mybir as mybir
import concourse.tile as tile
from concourse import bacc
from concourse.bass_utils import run_bass_kernel_spmd

P = 128
MMW = 512  # moving-operand slice width (one fp32 PSUM bank)

_BUILD_CACHE = {}


def build_attention_nc(T=2048, C=1024):
    key = (T, C)
    if key in _BUILD_CACHE:
        return _BUILD_CACHE[key]

    bf = mybir.dt.float16
    f8 = mybir.dt.float8e4
    f32 = mybir.dt.float32
    DR = mybir.MatmulPerfMode.DoubleRow
    NCC = C // P   # feature chunks (contraction)
    NT = T // P    # sequence chunks
    NJ = T // MMW  # moving slices per full row
    NH = C // MMW  # moving slices per V row
    TH = T // 2
    VW = C + P     # V tile width incl. ones column at [C] plus pad
    MSC = 8.0      # MT8 pre-scale (keeps M out of fp8 subnormal range)
    GS8 = 64.0     # host pre-scale on the fp8 G
    SCALE = 1.0 / float(np.sqrt(np.float32(C))) / MSC

    nc = bacc.Bacc("TRN2", debug=False)
    xT = nc.dram_tensor("xT", [C, T], bf, kind="ExternalInput").ap()
    x8d = nc.dram_tensor("x8", [C, T], f8, kind="ExternalInput").ap()
    gQ8 = nc.dram_tensor("gQ8", [NCC, P, C], f8, kind="ExternalInput").ap()
    wvT = nc.dram_tensor("wvT", [C, C], bf, kind="ExternalInput").ap()
    bs2 = nc.dram_tensor("bs2", [P, NT], f32, kind="ExternalInput").ap()
    bvB = nc.dram_tensor("bvB", [P, C], f32, kind="ExternalInput").ap()
    out = nc.dram_tensor("out", [T, C], f32, kind="ExternalOutput").ap()

    AF = mybir.ActivationFunctionType

    with tile.TileContext(nc) as tc:
        with (
            tc.tile_pool(name="consts", bufs=1) as consts,
            tc.tile_pool(name="qkv", bufs=1) as qkv,
            tc.tile_pool(name="small", bufs=4) as small,
        ):
            bs_t = consts.tile([P, NT], f32, tag="bs")
            bvb = consts.tile([P, C], f32, tag="bvb")
            # tri[p, f] = 1.0 where p <= f else 0.0 (diagonal mask block)
            tri = consts.tile([P, P], bf, tag="tri")
            nc.gpsimd.memset(tri[:], 1.0)
            nc.gpsimd.affine_select(
                out=tri[:], in_=tri[:],
                compare_op=mybir.AluOpType.is_ge, fill=0.0,
                base=0, pattern=[[1, P]], channel_multiplier=-1,
            )

            x_t = qkv.tile([P, NCC, T], bf, tag="x")
            x8 = qkv.tile([P, NCC, T], f8, tag="x8")
            MT8 = qkv.tile([P, NCC, T], f8, tag="MT8")
            VA = qkv.tile([P, NT, VW], bf, tag="VA")

            xT_r = xT.rearrange("(c p) t -> p c t", p=P)
            x8_r = x8d.rearrange("(c p) t -> p c t", p=P)
            wv_r = wvT.rearrange("(c p) o -> p c o", p=P)
            gQ8_r = gQ8.rearrange("c p o -> p c o")

            with tc.tile_pool(name="xv", bufs=1) as xv:
                wv_t = xv.tile([P, NCC, C], bf, tag="wv")

                # Ring 1 (sync): x8 in consumption order, then x (fp16).
                nc.sync.dma_start(out=x8[:, 0:NCC // 2, 0:MMW],
                                  in_=x8_r[:, 0:NCC // 2, 0:MMW])
                nc.sync.dma_start(out=x8[:, NCC // 2:NCC, 0:MMW],
                                  in_=x8_r[:, NCC // 2:NCC, 0:MMW])
                nc.sync.dma_start(out=x8[:, 0:NCC, MMW:2 * MMW],
                                  in_=x8_r[:, 0:NCC, MMW:2 * MMW])
                nc.sync.dma_start(out=x8[:, 0:NCC // 2, TH:T],
                                  in_=x8_r[:, 0:NCC // 2, TH:T])
                nc.sync.dma_start(out=x8[:, NCC // 2:NCC, TH:T],
                                  in_=x8_r[:, NCC // 2:NCC, TH:T])
                for h in range(2):
                    for c in range(NCC):
                        nc.sync.dma_start(
                            out=x_t[:, c, h * TH:(h + 1) * TH],
                            in_=xT_r[:, c, h * TH:(h + 1) * TH])

                with tc.tile_pool(name="xg", bufs=1) as xg:
                    g8_t = xg.tile([P, NCC, C], f8, tag="g8")
                    # Ring 2 (scalar): G8, biases, then wv.
                    for lo, hi in ((0, 2), (2, NCC // 2), (NCC // 2, NCC)):
                        nc.scalar.dma_start(out=g8_t[:, lo:hi, :],
                                            in_=gQ8_r[:, lo:hi, :])
                    nc.scalar.dma_start(out=bs_t[:], in_=bs2[:])
                    nc.scalar.dma_start(out=bvb[:], in_=bvB[:])
                    for c in range(0, NCC, NCC // 2):
                        nc.scalar.dma_start(out=wv_t[:, c:c + NCC // 2, :],
                                            in_=wv_r[:, c:c + NCC // 2, :])

                    # M projection, all fp8 DoubleRow: four T-quarter
                    # passes over eight 1-bank PSUM tiles; each arriving
                    # x8 piece feeds 8 matmuls. MT8 stores MSC*M.
                    with tc.tile_pool(name="psA", bufs=8,
                                      space="PSUM") as psA:
                        warm = psA.tile([P, MMW], f32, tag="pq", name="warm")
                        for k in range(12):
                            nc.tensor.matmul(warm[:, 0:P], tri[:], tri[:],
                                             start=(k == 0), stop=(k == 11))
                        for q in range(NJ):
                            psqs = [
                                psA.tile([P, MMW], f32, tag="pq",
                                         name=f"psq{q}_{m}")
                                for m in range(NCC)
                            ]
                            for cc in range(NCC // 2):
                                for m in range(NCC):
                                    nc.tensor.matmul(
                                        psqs[m][:],
                                        g8_t[:, 2 * cc:2 * cc + 2,
                                             m * P:(m + 1) * P],
                                        x8[:, 2 * cc:2 * cc + 2,
                                           q * MMW:(q + 1) * MMW],
                                        start=(cc == 0),
                                        stop=(cc == NCC // 2 - 1),
                                        perf_mode=DR,
                                    )
                            for m in range(NCC):
                                dst = MT8[:, m, q * MMW:(q + 1) * MMW]
                                if m % 2 == 0:
                                    nc.scalar.mul(dst, psqs[m][:],
                                                  MSC / GS8)
                                else:
                                    nc.vector.tensor_scalar_mul(
                                        dst, psqs[m][:], MSC / GS8)

                with (
                    tc.tile_pool(name="ptp", bufs=1) as ptp,
                    tc.tile_pool(name="outp", bufs=3) as outp,
                    tc.tile_pool(name="ps2", bufs=2, space="PSUM") as ps2,
                ):
                    PT = ptp.tile([P, NT, T], bf, tag="PT")

                    def scores_chunk(i, pss=None, rebase=None):
                        if pss is None:
                            pss = ps2.tile([P, T], f32, tag="ps", name="pss")
                        shift = 0 if rebase is None else rebase - i * P
                        jf = (i * P + MMW - 1) // MMW
                        slices = ([(i * P, jf * MMW - i * P)]
                                  if i * P < jf * MMW else [])
                        slices += [(j * MMW, MMW) for j in range(jf, NJ)]
                        for cc in range(NCC // 2):
                            for (off, w) in slices:
                                nc.tensor.matmul(
                                    pss[:, off + shift:off + shift + w],
                                    MT8[:, 2 * cc:2 * cc + 2,
                                        i * P:(i + 1) * P],
                                    x8[:, 2 * cc:2 * cc + 2, off:off + w],
                                    start=(cc == 0),
                                    stop=(cc == NCC // 2 - 1),
                                    perf_mode=DR,
                                )
                        nc.scalar.activation(
                            PT[:, i, i * P:T],
                            pss[:, i * P + shift:T + shift], AF.Exp,
                            bias=bs_t[:, i:i + 1], scale=SCALE,
                        )
                        nc.vector.tensor_mul(
                            PT[:, i, i * P:(i + 1) * P],
                            PT[:, i, i * P:(i + 1) * P],
                            tri[:],
                        )
                        return pss

                    # scores first: they only need x8/MT8/bias, so the
                    # x/wv streams for V still have ~60us of slack.
                    for i in range(NT - 2):
                        scores_chunk(i)
                    pss_tail = scores_chunk(NT - 2)
                    scores_chunk(NT - 1, pss=pss_tail, rebase=0)

                    # V projection (fp16), half-C PSUM tiles.
                    for n in range(NT):
                        for hh in range(NH):
                            psv = ps2.tile([P, MMW], f32, tag="ps",
                                           name=f"psv{n}_{hh}")
                            for c in range(NCC):
                                nc.tensor.matmul(
                                    psv[:],
                                    x_t[:, c, n * P:(n + 1) * P],
                                    wv_t[:, c, hh * MMW:(hh + 1) * MMW],
                                    start=(c == 0), stop=(c == NCC - 1),
                                )
                            nc.vector.tensor_add(
                                VA[:, n, hh * MMW:(hh + 1) * MMW],
                                psv[:], bvb[:, hh * MMW:(hh + 1) * MMW])
                        nc.vector.memset(VA[:, n, C:C + 1], 1.0)

                    def av_block(j):
                        pso = ps2.tile([P, C + MMW], f32, tag="ps",
                                       name="pso")
                        for i in range(j + 1):
                            pt_s = PT[:, i, j * P:(j + 1) * P]
                            for hh in range(NH):
                                nc.tensor.matmul(
                                    pso[:, hh * MMW:(hh + 1) * MMW],
                                    pt_s,
                                    VA[:, i, hh * MMW:(hh + 1) * MMW],
                                    start=(i == 0), stop=(i == j),
                                )
                            nc.tensor.matmul(
                                pso[:, C:C + 1],
                                pt_s,
                                VA[:, i, C:C + 1],
                                start=(i == 0), stop=(i == j),
                            )
                        rec = small.tile([P, 1], f32, tag="rec")
                        nc.vector.reciprocal(rec[:], pso[:, C:C + 1])
                        ot = outp.tile([P, C], f32, tag="ot")
                        for hh in range(NH):
                            s = hh * MMW
                            nc.scalar.mul(ot[:, s:s + MMW],
                                          pso[:, s:s + MMW], rec[:, 0:1])
                            nc.sync.dma_start(
                                out=out[j * P:(j + 1) * P, s:s + MMW],
                                in_=ot[:, s:s + MMW])

                    # j descending: tail ends on the 1-chunk block
                    for j in range(NT - 1, -1, -1):
                        av_block(j)

    nc.compile()
    _BUILD_CACHE[key] = nc
    return nc


def make_in_maps(x, wq, bq, wk, bk, wv, bv):
    """Host-side shard + layout prep. One in_map per core (= batch element)."""
    import ml_dtypes
    bfh = np.float16
    f8h = ml_dtypes.float8_e4m3
    x = np.asarray(x, dtype=np.float32)
    B, T, C = x.shape
    wq = np.asarray(wq, np.float32)
    wk = np.asarray(wk, np.float32)
    gTm = wq.T @ wk                                # [c_in, c_out] f32
    NCC = C // P
    # c-major fp8 G, pre-scaled by 64 so values (std ~0.013) land in
    # e4m3's normal range; the device copy divides the scale back out.
    gQ8k = np.ascontiguousarray(
        (gTm * np.float32(64.0)).astype(f8h).reshape(NCC, P, C))
    wvT = np.asarray(wv, np.float32).T.astype(bfh)
    v_b = wq.T @ np.asarray(bk, np.float32)        # [C]
    scale_div = np.float32(np.sqrt(np.float32(C)))
    bvf = np.ascontiguousarray(
        np.broadcast_to(np.asarray(bv, np.float32), (P, C)))
    in_maps = []
    for b in range(B):
        bs = (x[b] @ v_b) / scale_div              # [T] f32
        bs2 = np.ascontiguousarray(bs.reshape(T // P, P).T.astype(np.float32))
        xTb = np.ascontiguousarray(x[b].T)
        in_maps.append({
            "xT": xTb.astype(bfh),
            "x8": xTb.astype(f8h),
            "gQ8": gQ8k, "wvT": wvT,
            "bs2": bs2, "bvB": bvf,
        })
    return in_maps


def kernel(x, wq, bq, wk, bk, wv, bv):
    x = np.asarray(x, dtype=np.float32)
    B, T, C = x.shape
    nc = build_attention_nc(T, C)
    in_maps = make_in_maps(x, wq, bq, wk, bk, wv, bv)
    res = run_bass_kernel_spmd(nc, in_maps, core_ids=list(range(B)))
    out = np.stack([res.results[b]["out"] for b in range(B)], axis=0)[None]
    return np.ascontiguousarray(out.astype(np.float32))
